# revision 26
# baseline (speedup 1.0000x reference)
"""Trainium2 Bass kernel for nn_EvenOddFunctionHAM.

Computes, for W = W_tensor * W_mask (block-staircase 4096x4096):
    s_odd = rho(s) @ W + b_odd;  s_odd[:, :2048] += Ux
    out   = rho(s_odd) @ W.T + b_even
with rho(x) = sigmoid(4x - 2).

Strategy: data-parallel over the batch (4096 rows -> 8 cores x 512).
Everything runs in a transposed layout (feature dim on SBUF partitions,
batch on the free axis) so no on-device transposes are needed:
    S1 = W.T @ rho(s).T   (contraction over the even dim)
    O  = W  @ rho(S1+..)  (contraction over the odd dim)
Weights are masked, cast to bf16, transposed, and pre-tiled into
contiguous per-m-tile K-strips on the host; matmuls run in bf16 with
fp32 PSUM accumulation (bf16 over fp16: ~2% faster sustained, the PE is
power-limited and the narrower mantissa datapath draws less). The staircase zero block is skipped when the
masked W actually has it (checked on host), saving 25% of the FLOPs.
"""

import numpy as np

_KERNEL_CACHE = {}

_DEFAULT_OPTS = {
    "ring_split": True,
    "mm1_odd0_first": True,
    "psum_bufs": 8,
    # fuse_ldw=True re-fuses Ldweights into self-loading matmuls and enables
    # walrus --enable-ldw-opt. Measured perf-neutral on this kernel (the PE
    # stream is not LDW-bound), so keep the default, battle-tested compile
    # path.
    "fuse_ldw": False,
    "wpool_bufs": 3,
    "stage_bufs": 4,
    "strip_gpsimd": False,
    # Stream s.T / Ux.T as fp16: saves 6 MiB/core HBM traffic per rep.
    # With the bf16 datapath this measures 390.8 us vs 396-401 us (the PE
    # is power-limited; less DMA energy buys PE clock), rel err unchanged.
    "io_f16": True,
    # Timing probes (numerics-invalid when dup_mm > 1): emit each matmul
    # dup_mm times; strip_dup_ldw drops the duplicate Ldweights so the delta
    # measures raw LDW cost on hardware.
    "dup_mm": 1,
    "strip_dup_ldw": False,
    # Coalesce per-matmul progress-semaphore increments (~26 ns each,
    # serialized) into one multi-increment on each accumulation group's stop
    # matmul.
    "coalesce_updates": False,
    # max increments folded into one EventSemaphore (0 = unlimited)
    "coalesce_cap": 0,
    # bisect aid: only coalesce the first N groups (0 = all)
    "coalesce_first_n": 0,
    # compensate if sem-add-imm adds value+1
    "coalesce_minus1": False,
    # "pe": bulk add on PE queue after next MM; "sp": bulk add on the SP
    # queue gated by wait_ge(progress, post-stop count) — fires strictly
    # after the stop matmul's own drain-riding +1
    "coalesce_style": "sp",
    # write the output to HBM as bf16 (host upcasts to f32; halves out DMA)
    "out_bf16": False,
    # run the matmul datapath in bfloat16 instead of fp16 (same 1 cyc/row;
    # smaller mantissa multipliers draw less power under sustained load —
    # measured 396-401 us vs 406-411 us fp16 at R-diff(3,43); rel err
    # 2.5e-3 vs 3.1e-4, both far inside the 2e-2 gate)
    "mm_bf16": True,
}

# ---- model dims (hardcoded per contract; asserted against inputs) ----
B = 4096        # batch
E = 4096        # even dim (rows of W)
O_DIM = 4096    # odd dim (cols of W)
D1 = 2048       # width of Ux / first odd block
NC = 8          # cores
BC = B // NC    # batch per core = 512
P = 128         # partitions
NKE = E // P    # 32 k-tiles over even
NKO = O_DIM // P
NM1 = O_DIM // P  # mm1 output tiles (odd)
NM2 = E // P      # mm2 output tiles (even)
HALF = D1 // P    # 16


def _split_excess_waits(nc, maxw: int = 1) -> int:
    """This walrus build encodes at most one sem wait per instruction, but
    Tile's scheduler can attach several. Move the overflow onto inserted
    same-engine NoOps directly preceding the instruction (engines are
    in-order, so consecutive waits are equivalent to one multi-wait)."""
    from concourse import mybir

    n = 0
    for f in nc.m.functions:
        for bb in f.blocks:
            insts = bb.instructions
            new = []
            for inst in insts:
                si = getattr(inst, "sync_info", None)
                if si is not None and len(si.on_wait) > maxw:
                    waits = list(si.on_wait)
                    over, keep = waits[:-maxw], waits[-maxw:]
                    for j in range(0, len(over), maxw):
                        n += 1
                        new.append(mybir.InstNoOp(
                            name=f"{inst.name}-ws{j}",
                            engine=inst.engine,
                            bass_nofuse=True,
                            sync_info=mybir.SyncInfo(
                                on_wait=over[j : j + maxw], on_update=[]
                            ),
                        ))
                    inst.sync_info = mybir.SyncInfo(
                        on_wait=keep, on_update=list(si.on_update)
                    )
                new.append(inst)
            if len(new) != len(insts):
                insts[:] = new
                assert len(bb.instructions) == len(new)
    return n





def _coalesce_mm_updates_sp(nc, mybir, is_prog_inc, first_n: int = 0) -> int:
    """Strip the intermediate per-matmul +1s of each accumulation group and
    re-add them from the SP queue: an InstEventSemaphore that waits
    ge(progress, post-stop count) — i.e. strictly after the stop matmul's
    own drain-riding +1 has landed — then sem-add-imm's the group total.
    Wait thresholds are computed exactly during the walk (kept increments
    plus previously emitted bulks), so every consumer threshold is reached
    at or after its baseline time and the SP order keeps strip-DMA issues
    behind the group they were already gated on."""
    n_removed = 0
    n_groups = 0
    kept = 0      # +1s still attached to MMs, cumulative over the walk
    bulks = 0     # total re-added via SP EvSems emitted so far
    sem_id = None
    sem_type = None
    sem_name = None

    for f in nc.m.functions:
        for bb in f.blocks:
            insts = bb.instructions
            new = []
            run = []

            def strip(mms):
                nonlocal n_removed, sem_id, sem_type, sem_name
                total = 0
                for inst in mms:
                    si = inst.sync_info
                    u0 = next(u for u in si.on_update if is_prog_inc(u))
                    sem_id, sem_type, sem_name = u0.id, u0.sync_type, u0.ant_name
                    inst.sync_info = mybir.SyncInfo(
                        on_wait=list(si.on_wait),
                        on_update=[u for u in si.on_update
                                   if not is_prog_inc(u)])
                    total += 1
                n_removed += total
                return total

            for inst in insts:
                tn = type(inst).__name__
                si = getattr(inst, "sync_info", None)
                if tn == "InstMatmult":
                    ups = list(si.on_update) if si else []
                    incs = [u for u in ups if is_prog_inc(u)]
                    own = (not inst.stop_tensor_calc
                           and len(incs) == 1 and len(ups) == 1)
                    if own:
                        new.append(inst)
                        run.append(inst)
                        continue
                    # stop (or irregular) MM: keeps its own inc if it has one
                    new.append(inst)
                    if incs:
                        kept += 1
                    do = len(run) >= 2 and not (first_n and n_groups >= first_n)
                    if do and incs:
                        n_groups += 1
                        bulk = strip(run)
                        thresh = kept + bulks
                        nu = mybir.SyncUpdate(
                            sync_type=sem_type, id=sem_id,
                            update_mode="sem-add-imm", update_value=bulk,
                            ant_name=sem_name)
                        nw = mybir.SyncWait(
                            sync_type=sem_type, id=sem_id,
                            wait_mode="sem-ge-imm", wait_value=thresh,
                            ant_name=sem_name)
                        new.append(mybir.InstEventSemaphore(
                            name=f"{inst.name}-ci",
                            engine=mybir.EngineType.SP,
                            bass_nofuse=True,
                            sync_info=mybir.SyncInfo(
                                on_wait=[nw], on_update=[nu]),
                        ))
                        bulks += bulk
                    else:
                        # not coalescing: restore nothing, just count kept
                        kept += len(run)
                        run.clear()
                    run.clear()
                elif tn == "InstLdweights":
                    new.append(inst)
                else:
                    # leaving MM stream: any uncoalesced run keeps its incs
                    kept += len(run)
                    run.clear()
                    new.append(inst)
            kept += len(run)
            run.clear()
            insts[:] = new
    return n_removed

def _coalesce_mm_updates(nc, cap: int = 0, first_n: int = 0, minus1: bool = False, style: str = "sp") -> int:
    """Each matmul carries a +1 on Tile's PE progress semaphore; every
    increment is a serialized ~26 ns EVT_SEM write on the engine. Within an
    accumulation group (start..stop) the intermediate +1s are replaced by one
    InstEventSemaphore `sem-add-imm` of the run's total; the stop matmul
    keeps its own +1 (it rides the PSUM-drain pipeline and gates the ACT/DVE
    read of the accumulated bank).

    Placement: an EventSemaphore fires when the PE *engine* frees, but the
    preceding matmuls' trailing SBUF reads and PSUM writes drain ~173 ns
    later (PE_SBUF_ACCESS_LATENCY). The bulk update therefore goes after the
    NEXT matmul in the stream (>= 213 ns of separation); at stream
    boundaries a cycle-count NoOp provides the padding instead. Delaying by
    one matmul cannot deadlock: the only PE waits in that window (next
    strip's LDW, next group's psum-bank guard) depend on counts from >= 2
    groups earlier, whose EventSemaphores are long emitted."""
    from concourse import mybir

    def is_prog_inc(u):
        return (str(u.sync_type).endswith("semaphore")
                and u.update_mode == "sem-inc"
                and (u.update_value in (None, 1)))

    if style == "sp":
        return _coalesce_mm_updates_sp(nc, mybir, is_prog_inc, first_n)

    n_removed = 0
    n_groups = 0
    PAD_CYCLES = 300  # ~250 ns at the 1.2 GHz NX: covers the 173 ns drain

    for f in nc.m.functions:
        for bb in f.blocks:
            insts = bb.instructions
            new = []
            run = []       # current group's intermediate MMs
            pending = []   # EventSemaphores awaiting one-MM separation

            def make_ev(mms):
                nonlocal n_removed, n_groups
                if len(mms) < 2:
                    return None
                if first_n and n_groups >= first_n:
                    return None
                n_groups += 1
                u0 = None
                for inst in mms:
                    si = inst.sync_info
                    u0 = next(u for u in si.on_update if is_prog_inc(u))
                    inst.sync_info = mybir.SyncInfo(
                        on_wait=list(si.on_wait),
                        on_update=[u for u in si.on_update
                                   if not is_prog_inc(u)])
                n_removed += len(mms)
                nu = mybir.SyncUpdate(
                    sync_type=u0.sync_type, id=u0.id,
                    update_mode="sem-add-imm",
                    update_value=len(mms) - (1 if minus1 else 0),
                    ant_name=u0.ant_name)
                return mybir.InstEventSemaphore(
                    name=f"{mms[-1].name}-ci",
                    engine=mms[-1].engine,
                    bass_nofuse=True,
                    sync_info=mybir.SyncInfo(on_wait=[], on_update=[nu]),
                )

            def queue_flush():
                ev = make_ev(run)
                run.clear()
                if ev is not None:
                    pending.append(ev)

            def drain_pending(pad: bool):
                if not pending:
                    return
                if pad:
                    new.append(mybir.InstNoOp(
                        name=f"{pending[0].name}-pad",
                        engine=pending[0].engine,
                        bass_nofuse=True,
                        cycle_cnt=PAD_CYCLES,
                        sync_info=mybir.SyncInfo(on_wait=[], on_update=[]),
                    ))
                new.extend(pending)
                pending.clear()

            for inst in insts:
                tn = type(inst).__name__
                si = getattr(inst, "sync_info", None)
                if tn == "InstMatmult":
                    ups = list(si.on_update) if si else []
                    incs = [u for u in ups if is_prog_inc(u)]
                    own = (not inst.stop_tensor_calc
                           and len(incs) == 1 and len(ups) == 1)
                    if inst.stop_tensor_calc or not own:
                        new.append(inst)
                        queue_flush()        # bulk waits for next MM
                        continue
                    new.append(inst)
                    # this MM provides the >=213 ns separation for earlier evs
                    drain_pending(pad=False)
                    run.append(inst)
                    if cap and len(run) >= cap:
                        queue_flush()
                elif tn == "InstLdweights":
                    new.append(inst)  # transparent
                else:
                    # leaving the MM stream: pad with a cycle NoOp
                    queue_flush()
                    drain_pending(pad=True)
                    new.append(inst)
            queue_flush()
            drain_pending(pad=True)
            insts[:] = new
    return n_removed


def _strip_duplicate_ldweights(nc) -> int:
    """Drop an InstLdweights whose weight AP is identical to the immediately
    preceding one on the PE stream (the PE array still holds those weights).
    Waits from the dropped LDW move onto the following matmul."""
    from concourse import mybir

    n = 0
    for f in nc.m.functions:
        for bb in f.blocks:
            insts = bb.instructions
            new, last_key, pending = [], None, []
            for inst in insts:
                tn = type(inst).__name__
                if tn == "InstLdweights":
                    key = repr(inst.ins)
                    if key == last_key:
                        si = getattr(inst, "sync_info", None)
                        if si is not None:
                            pending += list(si.on_wait)
                            assert not si.on_update
                        n += 1
                        continue
                    last_key = key
                elif tn == "InstMatmult":
                    if pending:
                        si = getattr(inst, "sync_info", None)
                        waits = pending + list(si.on_wait if si else [])
                        ups = list(si.on_update if si else [])
                        inst.sync_info = mybir.SyncInfo(
                            on_wait=waits, on_update=ups)
                        pending = []
                new.append(inst)
            assert not pending
            if len(new) != len(insts):
                insts[:] = new
    return n


_LDW_PATCHED = False


def _patch_ldw_opt():
    """Compile with walrus --enable-ldw-opt=true (the concourse default
    pins it false). Requires self-loading matmuls (no explicit
    InstLdweights), which _fuse_ldweights produces."""
    global _LDW_PATCHED
    if _LDW_PATCHED:
        return
    from concourse import bass_utils
    _orig = bass_utils.run_command

    def _patched(argv, **kwargs):
        argv = ["--enable-ldw-opt=true" if a == "--enable-ldw-opt=false" else a
                for a in argv]
        return _orig(argv, **kwargs)

    bass_utils.run_command = _patched
    _LDW_PATCHED = True


def _fuse_ldweights(nc) -> int:
    """Tile legalization splits each matmul into InstLdweights + InstMatmult.
    Walrus's LDW optimization (fast weight load + pipelining) only applies to
    self-loading matmuls, so re-fuse: drop the Ldweights, move its sem waits
    onto the matmul, set ldweights=True."""
    from concourse import mybir

    n = 0
    for f in nc.m.functions:
        for bb in f.blocks:
            insts = bb.instructions
            new, pending = [], None
            for inst in insts:
                tn = type(inst).__name__
                if tn == "InstLdweights":
                    assert pending is None
                    pending = inst
                    continue
                if tn == "InstMatmult" and pending is not None:
                    si_l, si_m = pending.sync_info, inst.sync_info
                    waits = list(si_l.on_wait if si_l else []) + \
                        list(si_m.on_wait if si_m else [])
                    ups = list(si_l.on_update if si_l else []) + \
                        list(si_m.on_update if si_m else [])
                    inst.sync_info = mybir.SyncInfo(on_wait=waits, on_update=ups)
                    inst.ldweights = True
                    pending = None
                    n += 1
                new.append(inst)
            assert pending is None
            if len(new) != len(insts):
                insts[:] = new
    return n


def _build(sparse: bool, reps: int = 1, opts: dict | None = None, split_waits: bool = True):
    """Build the per-core Bass program (same program on all 8 cores).

    reps > 1 replicates the whole computation back-to-back inside one NEFF
    (output overwritten each rep) — used only for differential timing."""
    opts = dict(_DEFAULT_OPTS, **(opts or {}))
    import concourse.bass as bass
    import concourse.tile as tile
    from concourse import mybir

    f32 = mybir.dt.float32
    f16 = mybir.dt.bfloat16 if opts["mm_bf16"] else mybir.dt.float16

    nk1a = HALF if sparse else NKE   # mm1 K-tiles for odd0 m-tiles
    nk2b = HALF if sparse else NKO   # mm2 K-tiles for even1 m-tiles

    nc = bass.Bass("TRN2", target_bir_lowering=False, debug=False)

    io_dt = mybir.dt.float16 if opts["io_f16"] else f32
    sT = nc.dram_tensor("sT", [NKE, P, BC], io_dt, kind="ExternalInput")
    uT = nc.dram_tensor("uT", [HALF, P, BC], io_dt, kind="ExternalInput")
    w1a = nc.dram_tensor("w1a", [HALF, P, nk1a, P], f16, kind="ExternalInput")
    w1b = nc.dram_tensor("w1b", [HALF, P, NKE, P], f16, kind="ExternalInput")
    w2a = nc.dram_tensor("w2a", [HALF, P, NKO, P], f16, kind="ExternalInput")
    w2b = nc.dram_tensor("w2b", [HALF, P, nk2b, P], f16, kind="ExternalInput")
    bias1 = nc.dram_tensor("bias1", [P, NM1], f32, kind="ExternalInput")
    bias2 = nc.dram_tensor("bias2", [P, NM2], f32, kind="ExternalInput")
    out_dt = (mybir.dt.bfloat16 if opts["out_bf16"] else f32)
    out = nc.dram_tensor("o", [NM2, P, BC], out_dt, kind="ExternalOutput")

    with tile.TileContext(nc) as tc:
        with (
            tc.tile_pool(name="consts", bufs=1) as consts,
            tc.tile_pool(name="stage", bufs=opts["stage_bufs"]) as stage,
            tc.tile_pool(name="at", bufs=NKE) as at_pool,
            tc.tile_pool(name="ut", bufs=HALF) as ut_pool,
            tc.tile_pool(name="a2", bufs=NKO) as a2_pool,
            tc.tile_pool(name="wpool", bufs=opts["wpool_bufs"]) as wpool,
            tc.tile_pool(name="psum", bufs=opts["psum_bufs"], space="PSUM") as psum_pool,
            tc.tile_pool(name="opool", bufs=4) as opool,
        ):
            b1 = consts.tile([P, NM1], f32, tag="b1")
            nc.sync.dma_start(out=b1, in_=bias1[:, :])
            b2 = consts.tile([P, NM2], f32, tag="b2")
            nc.sync.dma_start(out=b2, in_=bias2[:, :])
            bneg2 = consts.tile([P, 1], f32, tag="bneg2")
            nc.vector.memset(bneg2, -2.0)

            pools = dict(
                stage=stage, at_pool=at_pool, ut_pool=ut_pool,
                a2_pool=a2_pool, wpool=wpool, psum_pool=psum_pool,
                opool=opool,
            )
            drams = dict(
                sT=sT, uT=uT, w1a=w1a, w1b=w1b, w2a=w2a, w2b=w2b, out=out
            )
            biases = dict(b1=b1, b2=b2, bneg2=bneg2)
            for _rep in range(reps):
                _kernel_body(nc, mybir, sparse, pools, drams, biases, opts)
    if opts["strip_dup_ldw"]:
        n = _strip_duplicate_ldweights(nc)
        print(f"stripped {n} duplicate Ldweights", flush=True)
    if opts["coalesce_updates"]:
        n = _coalesce_mm_updates(nc, cap=opts["coalesce_cap"], first_n=opts["coalesce_first_n"], minus1=opts["coalesce_minus1"], style=opts["coalesce_style"])
        print(f"coalesced {n} matmul sem increments", flush=True)
    if opts["fuse_ldw"]:
        _patch_ldw_opt()
        _fuse_ldweights(nc)
    if split_waits:
        _split_excess_waits(nc, 1)
    return nc


def _kernel_body(nc, mybir, sparse, pools, drams, biases, opts):
    f32 = mybir.dt.float32
    f16 = mybir.dt.bfloat16 if opts["mm_bf16"] else mybir.dt.float16
    nk1a = HALF if sparse else NKE
    nk2b = HALF if sparse else NKO
    stage, at_pool, ut_pool, a2_pool, wpool, psum_pool, opool = (
        pools["stage"], pools["at_pool"], pools["ut_pool"], pools["a2_pool"],
        pools["wpool"], pools["psum_pool"], pools["opool"],
    )
    sT, uT, w1a, w1b, w2a, w2b, out = (
        drams["sT"], drams["uT"], drams["w1a"], drams["w1b"], drams["w2a"],
        drams["w2b"], drams["out"],
    )
    b1, b2, bneg2 = biases["b1"], biases["b2"], biases["bneg2"]
    io_dt = mybir.dt.float16 if opts["io_f16"] else f32
    act_dma = nc.scalar if opts["ring_split"] else nc.sync
    strip_dma = nc.gpsimd if opts["strip_gpsimd"] else nc.sync

    # DMA ring split: weight strips go on the SP HWDGE ring (nc.sync),
    # activations/outputs on the ACT HWDGE ring (nc.scalar) — so the bulk
    # sT/uT loads never head-of-line-block the strip prefetch FIFO.

    # rho(s).T tiles, fp16, resident: AT[k] = sigmoid(4*sT[k] - 2)
    AT = []
    for k in range(NKE):
        st = stage.tile([P, BC], io_dt, tag="stage")
        act_dma.dma_start(out=st, in_=sT[k])
        a = at_pool.tile([P, BC], f16, tag="at")
        nc.scalar.activation(
            a, st, mybir.ActivationFunctionType.Sigmoid,
            bias=bneg2[:, 0:1], scale=4.0,
        )
        AT.append(a)

    # Ux.T tiles, fp32, resident
    UT = []
    for k in range(HALF):
        u = ut_pool.tile([P, BC], io_dt, tag="ut")
        act_dma.dma_start(out=u, in_=uT[k])
        UT.append(u)

    # ---- mm1: S1[odd,:] = W.T @ AT ; A2 = rho(S1 + b_odd [+ U]) ----
    # odd0 first: those m-tiles contract only over even0 (AT[0:16]), so the
    # PE can start after ~1/3 of the sT load instead of all of it.
    A2 = [None] * NM1
    mm1_order = (list(range(NM1)) if opts["mm1_odd0_first"]
                 else list(range(HALF, NM1)) + list(range(HALF)))
    for m in mm1_order:
        if m >= HALF:
            wt = wpool.tile([P, NKE, P], f16, tag="w")
            strip_dma.dma_start(out=wt, in_=w1b[m - HALF])
            ks = range(NKE)
        else:
            wt = wpool.tile([P, nk1a, P], f16, tag="w")
            strip_dma.dma_start(out=wt, in_=w1a[m])
            ks = range(nk1a)
        ps = psum_pool.tile([P, BC], f32, tag="ps")
        nkl = len(ks)
        dup = opts["dup_mm"]
        for i, k in enumerate(ks):
            for d in range(dup):
                nc.tensor.matmul(
                    ps, lhsT=wt[:, i, :], rhs=AT[k],
                    start=(i == 0 and d == 0),
                    stop=(i == nkl - 1 and d == dup - 1),
                )
        if m < HALF:
            nc.vector.tensor_add(ps, ps, UT[m])
        a2 = a2_pool.tile([P, BC], f16, tag="a2")
        nc.scalar.activation(
            a2, ps, mybir.ActivationFunctionType.Sigmoid,
            bias=b1[:, m : m + 1], scale=4.0,
        )
        A2[m] = a2

    # ---- mm2: O[even,:] = W @ A2 + b_even ----
    # even1 first (small strips, deps = A2[16:] = the tail of mm1).
    for m in list(range(HALF, NM2)) + list(range(HALF)):
        if m >= HALF:
            wt = wpool.tile([P, nk2b, P], f16, tag="w")
            strip_dma.dma_start(out=wt, in_=w2b[m - HALF])
            ks = range(NKO - nk2b, NKO)
        else:
            wt = wpool.tile([P, NKO, P], f16, tag="w")
            strip_dma.dma_start(out=wt, in_=w2a[m])
            ks = range(NKO)
        ps = psum_pool.tile([P, BC], f32, tag="ps")
        nkl = len(ks)
        dup = opts["dup_mm"]
        for i, k in enumerate(ks):
            for d in range(dup):
                nc.tensor.matmul(
                    ps, lhsT=wt[:, i, :], rhs=A2[k],
                    start=(i == 0 and d == 0),
                    stop=(i == nkl - 1 and d == dup - 1),
                )
        ot = opool.tile([P, BC],
                        f16 if opts["out_bf16"] else f32, tag="ot")
        nc.scalar.activation(
            ot, ps, mybir.ActivationFunctionType.Identity,
            bias=b2[:, m : m + 1], scale=1.0,
        )
        act_dma.dma_start(out=out[m], in_=ot)


def _strips(Wsub: np.ndarray, nm: int) -> np.ndarray:
    """[K, nm*128] -> [nm, 128, K//128, 128] contiguous per-m-tile K-strips.

    strip[j, p, kt, c] = Wsub[kt*128 + p, j*128 + c], so strip[j][:, kt, :]
    is the [K=128, M=128] lhsT tile for output tile j, contraction tile kt.
    """
    K = Wsub.shape[0]
    return np.ascontiguousarray(
        Wsub.reshape(K // P, P, nm, P).transpose(2, 1, 0, 3)
    )


def prepare_in_maps(inputs: dict, W: np.ndarray, sparse: bool, io_f16: bool = True,
                    mm_bf16: bool = False) -> list:
    """Host-side prep: mask+cast+tile weights, transpose activations, shard."""
    f32 = np.float32
    if mm_bf16:
        import ml_dtypes
        wdt = ml_dtypes.bfloat16
    else:
        wdt = np.float16
    s = np.asarray(inputs["s"], f32)
    Ux = np.asarray(inputs["Ux"], f32)
    assert s.shape == (B, E) and Ux.shape == (B, D1), (s.shape, Ux.shape)

    W16 = W.astype(wdt)
    WT16 = np.ascontiguousarray(W16.T)

    if sparse:
        w1a = _strips(W16[:D1, :D1], HALF)
        w2b = _strips(WT16[D1:, D1:], HALF)
    else:
        w1a = _strips(W16[:, :D1], HALF)
        w2b = _strips(WT16[:, D1:], HALF)
    w1b = _strips(W16[:, D1:], HALF)
    w2a = _strips(WT16[:, :D1], HALF)

    bias1 = np.ascontiguousarray(
        (4.0 * np.asarray(inputs["b_odd"], f32).reshape(-1) - 2.0).reshape(NM1, P).T
    )
    bias2 = np.ascontiguousarray(
        np.asarray(inputs["b_even"], f32).reshape(-1).reshape(NM2, P).T
    )

    io_dt = np.float16 if io_f16 else f32
    sT_full = np.ascontiguousarray(s.T.astype(io_dt))   # [E, B]
    uT_full = np.ascontiguousarray(Ux.T.astype(io_dt))  # [D1, B]

    in_maps = []
    for c in range(NC):
        sl = slice(c * BC, (c + 1) * BC)
        in_maps.append({
            "sT": np.ascontiguousarray(sT_full[:, sl]).reshape(NKE, P, BC),
            "uT": np.ascontiguousarray(uT_full[:, sl]).reshape(HALF, P, BC),
            "w1a": w1a, "w1b": w1b, "w2a": w2a, "w2b": w2b,
            "bias1": bias1, "bias2": bias2,
        })
    return in_maps


def kernel(Ux, s, W_tensor, b_even, b_odd, W_mask):
    from concourse.bass_utils import run_bass_kernel_spmd

    f32 = np.float32
    W = np.asarray(W_tensor, f32) * np.asarray(W_mask, f32)
    sparse = not W[D1:, :D1].any()

    in_maps = prepare_in_maps(
        {"s": s, "Ux": Ux, "b_odd": b_odd, "b_even": b_even}, W, sparse,
        io_f16=_DEFAULT_OPTS["io_f16"], mm_bf16=_DEFAULT_OPTS["mm_bf16"],
    )

    nc = _KERNEL_CACHE.get(sparse)
    if nc is None:
        nc = _build(sparse)
        _KERNEL_CACHE[sparse] = nc

    res = run_bass_kernel_spmd(nc, in_maps, core_ids=list(range(NC)))
    out_T = np.concatenate(
        [res.results[c]["o"].reshape(E, BC).astype(np.float32)
         for c in range(NC)], axis=1
    )  # [E, B]
    return np.ascontiguousarray(out_T.T)

